# revision 17
# baseline (speedup 1.0000x reference)
"""DFlashAttention (paged KV cache decode-attention block) on 8 Trainium2
NeuronCores.

Sharding: tensor-parallel over heads. Each core owns HQ/8 = 4 query heads and
HK/8 = 1 KV head (GQA group). Wq/Wk/Wv row-sharded, Wo column-sharded; each
core produces a partial output [B*S, HID] which is reduced on the host.

Device kernel layout (v2, bf16):
  - All heavy operands (x, weights, KV cache, probs, o) are bf16: full PE
    rate and half the HBM/DMA traffic of f32. PSUM accumulation stays f32.
  - Projections stream x + a fused [Wq|Wk|Wv] weight tensor in 8 groups of
    4 contraction chunks; the last group is emitted destination-major so K's
    norm chain starts ~20 matmuls before the last Q head finishes.
  - RMSNorm rstd chains are stage-batched (Square x5, Sqrt x5, ...); the
    RoPE half-swap is a PE matmul against a signed-permutation matrix with
    the partner-half norm weight folded in (no SBUF-SBUF swap DMAs), and
    per-token rstd broadcasts run on GpSimd partition_broadcast (no PE
    broadcast matmuls or PSUM round-trips).
  - Scores are computed transposed: [l_chunk(128), (head, s)=512] with the
    KV-cache chunk stationary, so softmax-sum reduces over partitions via a
    ones-matmul and PV consumes probs directly. The attention inner loop is
    software-pipelined two chunk-pair groups deep and flows across batch
    boundaries, so PE never waits on Exp.
  - Per-batch cache lengths are baked in at build time; the final partial
    cache chunk is masked by accumulating a rank-1 (-1e30) outer product.
  - KV cache loads are one DMA per batch (k on sync, v on scalar queue); Wo
    is prefetched during attention; output partials are staged to bf16 by
    DVE/GpSimd copies and stored on alternating queues.
"""

import sys

sys.path.insert(0, "/opt/trn_rl_repo")

import numpy as np
import ml_dtypes

BF = ml_dtypes.bfloat16
B, S, HID = 4, 128, 4096
D, HQ, HK = 128, 32, 8
PAGES, PSIZE, NPP = 64, 256, 16
THETA = 10000.0
EPS = 1e-6
N_CORES = 8
HQC = HQ // N_CORES  # 4 query heads per core
EC = HQC * D         # 512 output-proj contraction per core
BS = B * S           # 512 tokens
NDCH = HID // 128    # 32 contraction chunks for projections
GRP = 4              # d-chunks per projection DMA group

_CACHE = {}


def _prep_host(x, Wq, Wk, Wv, Wo, q_norm_w, k_norm_w, k_cache, v_cache,
               block_table, cache_seqlens):
    f32 = np.float32
    xT = np.ascontiguousarray(
        np.asarray(x, f32).reshape(BS, HID).T).astype(BF)

    lens = [int(v) for v in np.asarray(cache_seqlens)]
    pads = [(l + 127) // 128 * 128 for l in lens]
    offs = [0] * B
    for b in range(1, B):
        offs[b] = offs[b - 1] + pads[b - 1]
    total = max(sum(pads), 128)

    bt = np.asarray(block_table)
    kg = np.asarray(k_cache, f32)[bt].reshape(B, NPP * PSIZE, HK, D)
    vg = np.asarray(v_cache, f32)[bt].reshape(B, NPP * PSIZE, HK, D)

    # RoPE angles, range-reduced to [-pi, pi) on the host (index arithmetic
    # only; sin/cos evaluated on device). Mimic the reference's fp32 freqs.
    pos = np.asarray(cache_seqlens, np.float64)[:, None] + np.arange(S)[None, :]
    inv = 1.0 / (THETA ** (np.arange(0, D, 2, dtype=np.float64) / D))
    freqs32 = (pos.astype(f32)[:, :, None] * inv.astype(f32)[None, None, :]).astype(f32)
    fr = np.float64(freqs32)
    two_pi = 2 * np.pi

    def red(a):
        m = np.mod(a, two_pi)
        m = np.where(m >= np.pi, m - two_pi, m)
        return m.astype(f32)

    a_sin = red(fr).reshape(BS, 64).T                      # [64, BS]
    a_cos = red(fr + np.pi / 2).reshape(BS, 64).T
    # duplicate across both partition halves -> [128, BS]
    a_sin2 = np.ascontiguousarray(np.concatenate([a_sin, a_sin], 0))
    a_cos2 = np.ascontiguousarray(np.concatenate([a_cos, a_cos], 0))

    wq = np.asarray(q_norm_w, f32).reshape(D, 1)
    wk = np.asarray(k_norm_w, f32).reshape(D, 1)

    # Signed-permutation swap matrices with the partner-half weight folded
    # in: swapped[j] = sign(j) * w[(j+64)%128] * t[(j+64)%128],
    # sign = -1 for j < 64, +1 for j >= 64 (NeoX rotation).
    def mb(w):
        m = np.zeros((128, 128), f32)
        for j in range(128):
            i = (j + 64) % 128
            m[i, j] = (-1.0 if j < 64 else 1.0) * w[i, 0]
        return np.ascontiguousarray(m).astype(BF)

    MBq, MBk = mb(wq), mb(wk)

    # Per-partition exp bias: the masked tail positions of each batch's last
    # cache chunk live on the l-partition dim, so exp(score - 1e30) -> 0.
    maskb_h = np.zeros((128, B), f32)
    for b in range(B):
        t = lens[b] - (pads[b] // 128 - 1) * 128 if pads[b] > 0 else 128
        maskb_h[t:, b] = -1e30

    Wq_ = np.asarray(Wq, f32)
    Wk_ = np.asarray(Wk, f32)
    Wv_ = np.asarray(Wv, f32)
    Wo_ = np.asarray(Wo, f32)

    in_maps = []
    for c in range(N_CORES):
        wqkvT = np.ascontiguousarray(
            np.concatenate([Wq_[c * EC:(c + 1) * EC, :],
                            Wk_[c * D:(c + 1) * D, :],
                            Wv_[c * D:(c + 1) * D, :]], 0).T).astype(BF)
        woT = np.ascontiguousarray(Wo_[:, c * EC:(c + 1) * EC].T)
        kT = np.zeros((128, total), f32)
        vC = np.zeros((total, 128), f32)
        for b in range(B):
            nb, ob = lens[b], offs[b]
            if nb > 0:
                kT[:, ob:ob + nb] = kg[b, :nb, c, :].T
                vC[ob:ob + nb, :] = vg[b, :nb, c, :]
        vP = np.ascontiguousarray(
            vC.reshape(total // 128, 128, 128).transpose(1, 0, 2)
            .reshape(128, total))
        in_maps.append(dict(
            xT=xT, wqkvT=wqkvT, woT=woT,
            kT=np.ascontiguousarray(kT).astype(BF), vC=vP.astype(BF),
            a_sin=a_sin2, a_cos=a_cos2,
            wqA=wq, wkA=wk, MBq=MBq, MBk=MBk,
            epsq=np.full((1, 1), D * EPS, f32),
            epsk=np.full((1, 1), EPS, f32),
            maskb=maskb_h,
            ones=np.ones((128, 1), BF),
            ident=np.eye(128, dtype=f32).astype(BF),
        ))
    return in_maps, lens, pads, offs, total


def _build_nc(lens, pads, offs, total, reps=1, phases=3):
    import concourse.mybir as mybir
    import concourse.tile as tile
    from concourse import bacc

    F32 = mybir.dt.float32
    F32R = mybir.dt.float32r
    BF16 = mybir.dt.bfloat16
    AF = mybir.ActivationFunctionType
    OP = mybir.AluOpType

    nc = bacc.Bacc("TRN2", target_bir_lowering=False, debug=False,
                   num_devices=N_CORES)

    xT_d = nc.dram_tensor("xT", [HID, BS], BF16, kind="ExternalInput")
    wqkvT_d = nc.dram_tensor("wqkvT", [HID, 768], BF16, kind="ExternalInput")
    woT_d = nc.dram_tensor("woT", [EC, HID], F32R, kind="ExternalInput")
    kT_d = nc.dram_tensor("kT", [128, total], BF16, kind="ExternalInput")
    vC_d = nc.dram_tensor("vC", [128, total], BF16, kind="ExternalInput")
    asin_d = nc.dram_tensor("a_sin", [128, BS], F32, kind="ExternalInput")
    acos_d = nc.dram_tensor("a_cos", [128, BS], F32, kind="ExternalInput")
    wqA_d = nc.dram_tensor("wqA", [128, 1], F32, kind="ExternalInput")
    wkA_d = nc.dram_tensor("wkA", [128, 1], F32, kind="ExternalInput")
    MBq_d = nc.dram_tensor("MBq", [128, 128], BF16, kind="ExternalInput")
    MBk_d = nc.dram_tensor("MBk", [128, 128], BF16, kind="ExternalInput")
    epsq_d = nc.dram_tensor("epsq", [1, 1], F32, kind="ExternalInput")
    epsk_d = nc.dram_tensor("epsk", [1, 1], F32, kind="ExternalInput")
    maskb_d = nc.dram_tensor("maskb", [128, B], F32, kind="ExternalInput")
    ones_d = nc.dram_tensor("ones", [128, 1], BF16, kind="ExternalInput")
    id_d = nc.dram_tensor("ident", [128, 128], BF16, kind="ExternalInput")
    out_d = nc.dram_tensor("out", [BS, HID], BF16, kind="ExternalOutput")

    # DRAM big-views for batched loads: row-chunk c, partition p, col e.
    xT_v = xT_d.rearrange("(c p) e -> p c e", p=128)       # [128, 32, 512]
    wv_v = wqkvT_d.rearrange("(c p) e -> p c e", p=128)    # [128, 32, 768]
    woT_v = woT_d.rearrange("(c p) e -> p c e", p=128)     # [128, 4, 4096]

    with tile.TileContext(nc) as tc:
        with tc.tile_pool(name="const", bufs=1) as cpool, \
             tc.tile_pool(name="xpool", bufs=2) as xpool, \
             tc.tile_pool(name="wpool", bufs=2) as wpool, \
             tc.tile_pool(name="persist", bufs=1) as ppool, \
             tc.tile_pool(name="work", bufs=2) as wk, \
             tc.tile_pool(name="probp", bufs=4) as probp, \
             tc.tile_pool(name="wop", bufs=4) as wop, \
             tc.tile_pool(name="ps1", bufs=2, space="PSUM") as ps1, \
             tc.tile_pool(name="ps2", bufs=3, space="PSUM") as ps2:

            def body(_it):
                # ---- rope tables ----
                asin = wk.tile([128, BS], F32, tag="asin")
                nc.sync.dma_start(out=asin[:, :], in_=asin_d[:, :])
                acos = wk.tile([128, BS], F32, tag="acos")
                nc.sync.dma_start(out=acos[:, :], in_=acos_d[:, :])
                sin_t = ppool.tile([128, BS], BF16, tag="sin")
                nc.scalar.activation(sin_t[:, :], asin[:, :], AF.Sin)
                cos_t = ppool.tile([128, BS], BF16, tag="cos")
                nc.scalar.activation(cos_t[:, :], acos[:, :], AF.Sin)

                # ---- constants (DVE queue; DVE is idle in phase A) ----
                ones = cpool.tile([128, 1], BF16, tag="ones")
                nc.gpsimd.dma_start(out=ones[:, :], in_=ones_d[:, :])
                wqA = cpool.tile([128, 1], F32, tag="wqA")
                nc.gpsimd.dma_start(out=wqA[:, :], in_=wqA_d[:, :])
                wkA = cpool.tile([128, 1], F32, tag="wkA")
                nc.gpsimd.dma_start(out=wkA[:, :], in_=wkA_d[:, :])
                MBq = cpool.tile([128, 128], BF16, tag="MBq")
                nc.gpsimd.dma_start(out=MBq[:, :], in_=MBq_d[:, :])
                MBk = cpool.tile([128, 128], BF16, tag="MBk")
                nc.gpsimd.dma_start(out=MBk[:, :], in_=MBk_d[:, :])
                epsq = cpool.tile([1, 1], F32, tag="epsq")
                nc.gpsimd.dma_start(out=epsq[:, :], in_=epsq_d[:, :])
                epsk = cpool.tile([1, 1], F32, tag="epsk")
                nc.gpsimd.dma_start(out=epsk[:, :], in_=epsk_d[:, :])
                maskb = cpool.tile([128, B], F32, tag="maskb")
                nc.gpsimd.dma_start(out=maskb[:, :], in_=maskb_d[:, :])
                ident = cpool.tile([128, 128], BF16, tag="ident")
                nc.gpsimd.dma_start(out=ident[:, :], in_=id_d[:, :])

                # ---- phase A: Q,K,V projections in one streamed pass ----
                # psum alloc order chosen so attention's ps_s tiles reuse the
                # earliest-released buffers (q01 frees first in norm order).
                ps_q01 = ps2.tile([128, 1024], F32, tag="ps2", name="ps_q01")
                ps_q23 = ps2.tile([128, 1024], F32, tag="ps2", name="ps_q23")
                ps_kv = ps2.tile([128, 1024], F32, tag="ps2", name="ps_kv")
                ps_k = ps_kv[:, 0:512]
                ps_v = ps_kv[:, 512:1024]
                ps_qs = [ps_q01[:, 0:512], ps_q01[:, 512:1024],
                         ps_q23[:, 0:512], ps_q23[:, 512:1024]]
                # small first groups so PE starts quickly; last group emitted
                # destination-major so K (then V, q0, ...) completes early and
                # its norm chain overlaps the remaining projection matmuls.
                gsz = [2, 2] + [4] * ((NDCH - 4) // 4)
                gbase = [0] * len(gsz)
                for i in range(1, len(gsz)):
                    gbase[i] = gbase[i - 1] + gsz[i - 1]

                def wslice(wt, j, dest):
                    # dest: 0..3 = q heads, 4 = k, 5 = v
                    if dest < 4:
                        o = j * 768 + dest * 128
                    else:
                        o = j * 768 + 512 + (dest - 4) * 128
                    return wt[:, o:o + 128]

                for g, gn in enumerate(gsz):
                    c0 = gbase[g]
                    xtile = xpool.tile([128, GRP * BS], BF16, tag="xt",
                                       bufs=3)
                    nc.sync.dma_start(out=xtile[:, :gn * BS],
                                      in_=xT_v[:, c0:c0 + gn, :])
                    wt = wpool.tile([128, GRP * 768], BF16, tag="wt", bufs=3)
                    nc.scalar.dma_start(out=wt[:, :gn * 768],
                                        in_=wv_v[:, c0:c0 + gn, :])
                    if g < len(gsz) - 1:
                        for j in range(gn):
                            st = c0 + j == 0
                            xa = xtile[:, j * BS:(j + 1) * BS]
                            nc.tensor.matmul(ps_k, wslice(wt, j, 4), xa,
                                             start=st, stop=False)
                            nc.tensor.matmul(ps_v, wslice(wt, j, 5), xa,
                                             start=st, stop=False)
                            for h in range(HQC):
                                nc.tensor.matmul(ps_qs[h], wslice(wt, j, h),
                                                 xa, start=st, stop=False)
                    else:
                        for dest, pdst in ((4, ps_k), (5, ps_v),
                                           (0, ps_qs[0]), (1, ps_qs[1]),
                                           (2, ps_qs[2]), (3, ps_qs[3])):
                            for j in range(gn):
                                xa = xtile[:, j * BS:(j + 1) * BS]
                                nc.tensor.matmul(pdst,
                                                 wslice(wt, j, dest), xa,
                                                 start=False,
                                                 stop=j == gn - 1)

                # ---- KV cache + Wo loads: scalar queue, strictly behind the
                # weight stream so they never steal DMA engines from phase A;
                # they flow during norms/attention. ----
                kT_all = ppool.tile([128, total], BF16, tag="kT_all")
                vC_all = ppool.tile([128, total], BF16, tag="vC_all")
                for b in range(B):
                    o0, o1 = offs[b], offs[b] + pads[b]
                    nc.scalar.dma_start(out=kT_all[:, o0:o1],
                                        in_=kT_d[:, o0:o1])
                    nc.scalar.dma_start(out=vC_all[:, o0:o1],
                                        in_=vC_d[:, o0:o1])
                wos = []
                for q in range(4):
                    wo = wop.tile([128, 4096], F32R, tag="wo", name=f"wo{q}",
                                  bufs=2)
                    nc.scalar.dma_start(
                        out=wo[:, :],
                        in_=woT_v[:, :, q * 1024:(q + 1) * 1024])
                    wos.append(wo)

                # ---- norms for k + 4 q heads, stage-batched ----
                heads = [(ps_k, wkA, MBk, epsk, 1.0 / D)] + \
                        [(ps_qs[h], wqA, MBq, epsq, 1.0) for h in range(HQC)]
                sqs, tsbs, pss, swaps, sqrs, rstds, bcs = [], [], [], [], [], [], []
                for i, (ps_in, _, _, _, _) in enumerate(heads):
                    sq = wk.tile([128, BS], BF16, tag="sq", name=f"sq{i}",
                                 bufs=5)
                    nc.scalar.activation(sq[:, :], ps_in, AF.Square)
                    sqs.append(sq)
                for i in range(5):
                    ps_ss = ps1.tile([1, BS], F32, tag="ps1", name=f"ss{i}")
                    nc.tensor.matmul(ps_ss[:, :], ones[:, :], sqs[i][:, :],
                                     start=True, stop=True)
                    pss.append(ps_ss)
                for i in range(5):
                    sqr = wk.tile([1, BS], F32, tag="sqr", name=f"sqr{i}",
                                  bufs=5)
                    nc.scalar.activation(sqr[:, :], pss[i][:, :], AF.Sqrt,
                                         scale=heads[i][4],
                                         bias=heads[i][3][:, :])
                    sqrs.append(sqr)
                # v_tmp + t_sb copies on ACT (PSUM -> bf16 SBUF)
                v_tmp = ppool.tile([128, BS], BF16, tag="v_tmp")
                nc.scalar.activation(v_tmp[:, :], ps_v, AF.Copy)
                for i, (ps_in, _, _, _, _) in enumerate(heads):
                    t_sb = wk.tile([128, BS], BF16, tag="t_sb",
                                   name=f"tsb{i}", bufs=5)
                    nc.scalar.activation(t_sb[:, :], ps_in, AF.Copy)
                    tsbs.append(t_sb)
                for i in range(5):
                    ps_sw = ps1.tile([128, BS], F32, tag="ps1",
                                     name=f"sw{i}")
                    nc.tensor.matmul(ps_sw[:, :], heads[i][2][:, :],
                                     tsbs[i][:, :], start=True, stop=True)
                    swaps.append(ps_sw)
                for i in range(5):
                    rstd = wk.tile([1, BS], BF16, tag="rstd", name=f"rstd{i}",
                                   bufs=5)
                    with nc.allow_low_precision(reason="bf16 rounding"):
                        nc.vector.reciprocal(rstd[:, :], sqrs[i][:, :])
                    rstds.append(rstd)
                for i in range(5):
                    bc = wk.tile([128, BS], BF16, tag="bc", name=f"bc{i}",
                                 bufs=5)
                    nc.gpsimd.partition_broadcast(bc[:, :], rstds[i][:, :],
                                                  channels=128)
                    bcs.append(bc)
                k_sb = ppool.tile([128, BS], BF16, tag="k_sb")
                q_sb = ppool.tile([128, HQC * BS], BF16, tag="q_sb")
                dsts = [k_sb[:, :]] + [q_sb[:, h * BS:(h + 1) * BS]
                                       for h in range(HQC)]
                for i in range(5):
                    _, wA, _, _, _ = heads[i]
                    m1 = wk.tile([128, BS], BF16, tag="m1", bufs=2)
                    nc.vector.scalar_tensor_tensor(
                        m1[:, :], tsbs[i][:, :], wA[:, :], cos_t[:, :],
                        op0=OP.mult, op1=OP.mult)
                    m2 = wk.tile([128, BS], BF16, tag="m2", bufs=2)
                    nc.vector.tensor_mul(m2[:, :], swaps[i][:, :],
                                         sin_t[:, :])
                    rt = wk.tile([128, BS], BF16, tag="rt", bufs=2)
                    nc.vector.tensor_add(rt[:, :], m1[:, :], m2[:, :])
                    nc.vector.tensor_mul(dsts[i], rt[:, :], bcs[i][:, :])

                # ---- v: transpose to [token, D] per batch -> vt_sb ----
                vt_sb = ppool.tile([128, BS], BF16, tag="vt_sb")
                for b in range(B):
                    ps_vt = ps1.tile([128, 128], BF16, tag="ps1",
                                     name=f"ps_vt{b}")
                    nc.tensor.transpose(ps_vt[:, :],
                                        v_tmp[:, b * S:(b + 1) * S],
                                        ident[:, :])
                    nc.vector.tensor_copy(vt_sb[:, b * S:(b + 1) * S],
                                          ps_vt[:, :])

                # q viewed as [128, h, b, s] for per-batch 3D moving operands
                q4 = q_sb.rearrange("p (h b s) -> p h b s", h=HQC, b=B)

                # ---- attention: software-pipelined across chunk-pair groups
                # AND batch boundaries (pending depth 2) ----
                o_sb = ppool.tile([128, B * 512], F32R, tag="o_sb")

                def kchunk(b, ci):
                    if ci == pads[b] // 128:
                        return k_sb[:, b * S:(b + 1) * S]
                    o = offs[b] + ci * 128
                    return kT_all[:, o:o + 128]

                def vchunk(b, ci):
                    if ci == pads[b] // 128:
                        return vt_sb[:, b * S:(b + 1) * S]
                    o = offs[b] + ci * 128
                    return vC_all[:, o:o + 128]

                ps_os, ps_sums = {}, {}
                pend = []

                def emit_flush(e):
                    b, gi, ngr = e["b"], e["gi"], e["ngr"]
                    first = gi == 0
                    last = gi == ngr - 1
                    if first:
                        ps_os[b] = ps1.tile([128, 512], F32, tag="ps1",
                                            name=f"ps_o{b}")
                        ps_sums[b] = ps1.tile([1, 512], F32, tag="ps1",
                                              name=f"ps_sum{b}")
                    ps_o, ps_sum = ps_os[b], ps_sums[b]
                    grp = e["cis"]
                    for k, ci in enumerate(grp):
                        st = first and k == 0
                        sp = last and k == len(grp) - 1
                        pr = e["prob"][:, k * 512:(k + 1) * 512]
                        nc.tensor.matmul(ps_o[:, :], vchunk(b, ci), pr,
                                         start=st, stop=sp)
                        nc.tensor.matmul(ps_sum[:, :], ones[:, :], pr,
                                         start=st, stop=sp)
                    if last:
                        rec = wk.tile([1, 512], BF16, tag="rec", bufs=2)
                        with nc.allow_low_precision(reason="bf16 rounding"):
                            nc.vector.reciprocal(rec[:, :], ps_sum[:, :])
                        bco = wk.tile([128, 512], BF16, tag="bco", bufs=2)
                        nc.gpsimd.partition_broadcast(bco[:, :], rec[:, :],
                                                      channels=128)
                        nc.vector.tensor_mul(o_sb[:, b * 512:(b + 1) * 512],
                                             ps_o[:, :], bco[:, :])

                for b in range(B):
                    ncache = pads[b] // 128
                    tail = lens[b] - (ncache - 1) * 128 if ncache > 0 else 0
                    cis = list(range(ncache + 1))
                    groups = [cis[i:i + 2] for i in range(0, len(cis), 2)]
                    for gi, grp in enumerate(groups):
                        ps_s = ps2.tile([128, 1024], F32, tag="ps2",
                                        name=f"ps_s{b}_{gi}")
                        for k, ci in enumerate(grp):
                            if b == 0 and gi < 8:
                                # per-head score matmuls: head h only needs
                                # its own rope'd q, so early heads start
                                # during the norm tail of later heads.
                                for h in range(HQC):
                                    nc.tensor.matmul(
                                        ps_s[:, k * 512 + h * S:
                                             k * 512 + (h + 1) * S],
                                        kchunk(b, ci), q4[:, h, b, :],
                                        start=True, stop=True)
                            else:
                                # full-width: one Ldweights per chunk
                                nc.tensor.matmul(
                                    ps_s[:, k * 512:(k + 1) * 512],
                                    kchunk(b, ci), q4[:, :, b, :],
                                    start=True, stop=True)
                        prob = probp.tile([128, 1024], BF16, tag="prob")
                        halves = [(k, ci < ncache and ci == ncache - 1
                                   and tail < 128)
                                  for k, ci in enumerate(grp)]
                        if any(m for _, m in halves):
                            for k, m in halves:
                                nc.scalar.activation(
                                    prob[:, k * 512:(k + 1) * 512],
                                    ps_s[:, k * 512:(k + 1) * 512], AF.Exp,
                                    bias=(maskb[:, b:b + 1] if m else 0.0))
                        else:
                            width = 512 * len(grp)
                            nc.scalar.activation(prob[:, 0:width],
                                                 ps_s[:, 0:width], AF.Exp)
                        pend.append(dict(b=b, gi=gi, ngr=len(groups),
                                         cis=grp, prob=prob))
                        if len(pend) > 2:
                            emit_flush(pend.pop(0))
                while pend:
                    emit_flush(pend.pop(0))

                # ---- output projection: partial = o @ WoT_c ----
                for quarter in range(4):
                    wo = wos[quarter]
                    for b in range(B):
                        ps_out = ps2.tile([128, 1024], F32, tag="ps2",
                                          name=f"ps_out{quarter}_{b}")
                        for hc in range(2):
                            for h in range(HQC):
                                nc.tensor.matmul(
                                    ps_out[:, hc * 512:(hc + 1) * 512],
                                    o_sb[:, b * 512 + h * D:
                                         b * 512 + (h + 1) * D],
                                    wo[:, h * 1024 + hc * 512:
                                       h * 1024 + (hc + 1) * 512],
                                    start=(h == 0), stop=(h == HQC - 1))
                        od = wk.tile([128, 1024], BF16, tag="od", bufs=4)
                        idx = quarter * B + b
                        if idx % 2 == 0:
                            nc.vector.tensor_copy(od[:, :], ps_out[:, :])
                        else:
                            nc.scalar.activation(od[:, :], ps_out[:, :],
                                                 AF.Copy)
                        eng = nc.sync if idx % 2 == 0 else nc.scalar
                        eng.dma_start(
                            out=out_d[b * S:(b + 1) * S,
                                      quarter * 1024:(quarter + 1) * 1024],
                            in_=od[:, :])

            if reps == 1:
                body(0)
            else:
                with tc.For_i(0, reps, 1,
                              hint_engines=(mybir.EngineType.PE,
                                            mybir.EngineType.Activation,
                                            mybir.EngineType.Pool,
                                            mybir.EngineType.DVE,
                                            mybir.EngineType.SP)) as it:
                    body(it)

    nc.compile()
    return nc


def _get_nc(lens, pads, offs, total, reps=1, phases=3):
    key = (tuple(lens), total, reps, phases)
    if key not in _CACHE:
        _CACHE[key] = _build_nc(lens, pads, offs, total, reps, phases)
    return _CACHE[key]


def kernel(x, Wq, Wk, Wv, Wo, q_norm_w, k_norm_w, k_cache, v_cache,
           block_table, cache_seqlens):
    from concourse.bass_utils import run_bass_kernel_spmd

    in_maps, lens, pads, offs, total = _prep_host(
        x, Wq, Wk, Wv, Wo, q_norm_w, k_norm_w, k_cache, v_cache,
        block_table, cache_seqlens)
    nc = _get_nc(lens, pads, offs, total, reps=1)
    res = run_bass_kernel_spmd(nc, in_maps, core_ids=list(range(N_CORES)))
    partials = np.stack([np.asarray(r["out"], np.float32)
                         for r in res.results], 0)
    out = np.sum(partials, axis=0, dtype=np.float64).astype(np.float32)
    return out.reshape(B, S, HID)
